# revision 22
# baseline (speedup 1.0000x reference)
"""Trainium2 Bass kernel for nn_CA1AttentionGate (two-program, zero-collective).

Reference computation (B=1, S=8192, H=1024, F=128, K=2):
    temporal = relu(t @ Wt1 + bt1) @ Wt2 + bt2          [K,F]
    mem      = dg_features + temporal                    [K,F]
    qmean    = query.mean(axis=1)                        [1,H]
    score_k  = tanh([mem_k ; qmean] @ Wa1 + ba1) @ Wa2 + ba2
    w_k      = sigmoid(score_k)
    g_k      = mem_k @ Wg + bg                           [K,H]
    row[s]   = (1/K) * sum_k w_k * (g_k . key[s])        [S]
    out      = broadcast(row) -> [1,1,S,S]

Sharding: sequence-parallel across 8 cores.  Each core owns 1024 positions
of both query (for the qmean reduction) and key (for the gate row), and
produces only its 1024-wide slice of the broadcast row; the [S,S] broadcast
is a zero-copy numpy view at gather time (every row is identical).

Cross-core structure: the only global quantity is qmean.  A collective
would cost ~15us of modelled latency, so the kernel runs as TWO
back-to-back SPMD programs with a pure data-marshalling hop on the host:

  P1 (per core): read its query shard quantized to fp8 in a host-transposed
      h-major layout and reduce it over the sequence axis on DVE+ACT into
      per-column partial sums qmT[p, c] = sum_s q[s, 8p+c] (f32, 4KB out).
      (The PE would be 2.5x faster per element, but the cost model's
      p-state ramp pins DMA-gated matmul trains at the 1.2GHz MID clock,
      which makes the vector engines the better reducers here.)
  host: restack the 8 partial tiles (no arithmetic) into P2's input pack.
  P2 (per core): finish qmean + the scorer MLP -> w_k on device, build the
      combined gate vector G = sum_k w_k g_k (+ (sum w_k) bg), then do the
      matvec row[s] = G . key[s] over the host-pre-transposed f16 key
      shard on the PE, and write the 4KB row slice.

Quantization (tolerance is 2e-2; measured end-to-end error ~5e-4):
  query fp8 (qmean averages the noise down by sqrt(8192)), key/Wg/Wa1m
  f16, Wa1q fp8 x qmean fp8 (qmean pre-scaled by 64 so its ~0.01-scale
  values clear the fp8 subnormal floor; the 1/64 is folded back in the
  +ba1 bias step).  The output row stays f32.
"""

import numpy as np

SEQ = 8192
H = 1024
F = 128
K = 2
NCORES = 8
SHARD = SEQ // NCORES  # 1024
HC = H // 128  # 8 h-chunks

_PROG_CACHE = {}

QM_SCALE = 64.0  # qmean pre-scale feeding the fp8 hq matmuls

# f32 pack column layout (P2): qpc | small weights
_C_QPC = 0  # 64 cols: qpc[p, c*8+d] = qmT_d[p, c]
_C_BT2 = 64
_C_BA1 = 65
_C_WA2 = 66
_C_DGT = 67  # 2 cols
_C_BA2 = 69  # row 0 only
_C_TB = 70  # 2 cols, rows 0:32
_C_WT1 = 72  # rows 0:32
_C_BT1 = 73  # rows 0:32
_C_BGT = 74  # 8 cols
_C_WT2 = 82  # 128 cols, rows 0:32
_F32_COLS = 210

# f16 pack column layout (P2)
_C_WG = 0  # 1024 cols
_C_WA1M = 1024  # 128 cols
_F16_COLS = 1152

# P1 chunking: c-columns per DMA chunk (small first chunk for an early
# engine start, small last chunk for a short tail)
_P1_CHUNKS = (1, 2, 2, 2, 1)
# P2 kT chunking
_P2_CHUNKS = (3, 2, 2, 1)


def _build_p1():
    """P1: query-shard column sums on DVE+ACT.

    in : qT  fp8 [128, HC, SHARD]  (qT[p, c, s] = q_shard[s, 8p + c])
    out: qmT f32 [128, HC]         (qmT[p, c] = sum_s q_shard[s, 8p + c])
    """
    import concourse.bacc as bacc
    import concourse.tile as tile
    from concourse import mybir

    AF = mybir.ActivationFunctionType
    ALU = mybir.AluOpType
    f32 = mybir.dt.float32
    f8 = mybir.dt.float8e4

    nc = bacc.Bacc("TRN2", target_bir_lowering=False, debug=False,
                   num_devices=NCORES)

    qT = nc.dram_tensor("qT", [128, HC, SHARD], f8, kind="ExternalInput").ap()
    qmT_d = nc.dram_tensor("qmT", [128, HC], f32, kind="ExternalOutput").ap()

    with tile.TileContext(nc) as tc:
        with (
            tc.tile_pool(name="consts", bufs=1) as cp,
            tc.tile_pool(name="qstream", bufs=1) as qp,
        ):
            # --- stream the transposed query shard (sync queue) -----------
            qtiles = []
            pos = 0
            for i, w in enumerate(_P1_CHUNKS):
                qt = qp.tile([128, w * SHARD], f8, tag=f"q{i}")
                nc.sync.dma_start(
                    qt, qT.rearrange("p c s -> p (c s)")[:, pos * SHARD : (pos + w) * SHARD]
                )
                qtiles.append((qt, pos, w))
                pos += w

            out_sb = cp.tile([128, HC], f32, tag="qmT")

            # --- per-column sequence reduction on DVE + ACT ---------------
            def col_ap(c):
                for qt, pos, w in qtiles:
                    if pos <= c < pos + w:
                        return qt[:, (c - pos) * SHARD : (c - pos + 1) * SHARD]
                raise AssertionError

            junk = cp.tile([128, SHARD], f32, tag="junk")
            # DVE is the faster reducer (no accumulator-read tax), so it
            # takes 4 full columns + the first half of c7; ACT takes 3 full
            # columns + the second half of c7.
            for c in (0, 2, 4, 6):
                nc.vector.tensor_reduce(out_sb[:, c : c + 1], col_ap(c),
                                        axis=mybir.AxisListType.X, op=ALU.add)
            for c in (1, 3, 5):
                nc.scalar.activation(junk, col_ap(c), AF.Copy,
                                     accum_out=out_sb[:, c : c + 1])
            hparts = cp.tile([128, 2], f32, tag="hparts")
            src7 = col_ap(7)
            nc.vector.tensor_reduce(hparts[:, 0:1], src7[:, 0 : SHARD // 2],
                                    axis=mybir.AxisListType.X, op=ALU.add)
            nc.scalar.activation(junk[:, 0 : SHARD // 2],
                                 src7[:, SHARD // 2 : SHARD], AF.Copy,
                                 accum_out=hparts[:, 1:2])
            nc.vector.tensor_add(out_sb[:, 7:8], hparts[:, 0:1], hparts[:, 1:2])

            nc.gpsimd.dma_start(qmT_d, out_sb)

    nc.compile()
    return nc


def _build_p2():
    """P2: qmean finish + scorer + gate row matvec.

    in : wpk f16 [128, 64]         (ones: PE/ACT warmup fodder)
         fpk f32 [128, _F32_COLS]  (qpc + packed small weights, see _C_*)
         apk fp8 [128, HC, F]      (Wa1[F:F+H] rows h=8i+c -> [i, c, f])
         hpk f16 [128, _F16_COLS]  (Wg/K | Wa1m)
         kT  f16 [128, HC, SHARD]  (kT[p, c, s] = key[d*SHARD+s, c*128+p])
    out: orow f32 [SHARD]
    """
    import concourse.bacc as bacc
    import concourse.tile as tile
    from concourse import mybir

    AF = mybir.ActivationFunctionType
    ALU = mybir.AluOpType
    f32 = mybir.dt.float32
    f16 = mybir.dt.float16
    f8 = mybir.dt.float8e4

    nc = bacc.Bacc("TRN2", target_bir_lowering=False, debug=False,
                   num_devices=NCORES)

    wpk = nc.dram_tensor("wpk", [128, 64], f16, kind="ExternalInput").ap()
    fpk = nc.dram_tensor("fpk", [128, _F32_COLS], f32, kind="ExternalInput").ap()
    apk = nc.dram_tensor("apk", [128, HC, F], f8, kind="ExternalInput").ap()
    hpk = nc.dram_tensor("hpk", [128, _F16_COLS], f16, kind="ExternalInput").ap()
    kT = nc.dram_tensor("kT", [128, HC, SHARD], f16, kind="ExternalInput").ap()
    orow = nc.dram_tensor("orow", [SHARD], f32, kind="ExternalOutput").ap()

    with tile.TileContext(nc) as tc:
        with (
            tc.tile_pool(name="consts", bufs=1) as cp,
            tc.tile_pool(name="work", bufs=1) as wp,
            tc.tile_pool(name="ps_small", bufs=1, space="PSUM") as pps,
            tc.tile_pool(name="ps_big", bufs=1, space="PSUM") as ppb,
        ):
            # --- the wire: tiny warmup fodder first, then fpk (feeds the
            # longest scorer chain, so it rides the ACT queue to land right
            # after wpk), then hpk/apk and the key stream on the sync queue
            wpk_sb = cp.tile([128, 64], f16, tag="wpk")
            nc.sync.dma_start(wpk_sb, wpk)
            fpk_sb = cp.tile([128, _F32_COLS], f32, tag="fpk")
            nc.scalar.dma_start(fpk_sb, fpk)
            hpk_sb = cp.tile([128, _F16_COLS], f16, tag="hpk")
            nc.sync.dma_start(hpk_sb, hpk)
            apk_sb = cp.tile([128, HC, F], f8, tag="apk")
            nc.sync.dma_start(apk_sb, apk)
            ktiles = []
            pos = 0
            for i, w in enumerate(_P2_CHUNKS):
                kt = cp.tile([128, w * SHARD], f16, tag=f"k{i}")
                nc.sync.dma_start(
                    kt, kT.rearrange("p c s -> p (c s)")[:, pos * SHARD : (pos + w) * SHARD]
                )
                ktiles.append((kt, pos, w))
                pos += w

            def k_ap(c, hh):
                for kt, p0, w in ktiles:
                    if p0 <= c < p0 + w:
                        base = (c - p0) * SHARD + hh * 512
                        return kt[:, base : base + 512]
                raise AssertionError

            # pack slices
            Wg_sb = hpk_sb[:, _C_WG : _C_WG + H]
            Wa1m_sb = hpk_sb[:, _C_WA1M : _C_WA1M + F]
            ones16 = wpk_sb[:, 0:1]
            warm_sb = wpk_sb[:, :]
            qpc_sb = fpk_sb[:, _C_QPC : _C_QPC + 64]
            bt2T_sb = fpk_sb[:, _C_BT2 : _C_BT2 + 1]
            ba1T_sb = fpk_sb[:, _C_BA1 : _C_BA1 + 1]
            Wa2_sb = fpk_sb[:, _C_WA2 : _C_WA2 + 1]
            dgT_sb = fpk_sb[:, _C_DGT : _C_DGT + K]
            ba2b_sb = fpk_sb[0:1, _C_BA2 : _C_BA2 + 1]
            bgT_sb = fpk_sb[:, _C_BGT : _C_BGT + HC]
            tb_sb = fpk_sb[0:32, _C_TB : _C_TB + K]
            Wt1T_sb = fpk_sb[0:32, _C_WT1 : _C_WT1 + 1]
            bt1T_sb = fpk_sb[0:32, _C_BT1 : _C_BT1 + 1]
            Wt2_sb = fpk_sb[0:32, _C_WT2 : _C_WT2 + F]

            # --- ACT table warmup for the late tanh/sigmoid ---------------
            w1 = cp.tile([1, 1], f32, tag="w1")
            nc.scalar.activation(w1, wpk_sb[0:1, 0:1], AF.Tanh)
            w2 = cp.tile([1, 1], f32, tag="w2")
            nc.scalar.activation(w2, w1, AF.Sigmoid)

            # --- PE p-state warmup fodder ---------------------------------
            wjunk = pps.tile([1, 64], f32, tag="wjunk")
            for _ in range(10):
                nc.tensor.matmul(wjunk, lhsT=ones16, rhs=warm_sb,
                                 start=True, stop=True)

            # --- qmean columns (x QM_SCALE/SEQ, in fp8 for the hq matmuls)
            qmT = wp.tile([128, HC], f32, tag="qmT")
            nc.vector.tensor_reduce(
                qmT, qpc_sb.rearrange("p (c d) -> p c d", c=HC),
                axis=mybir.AxisListType.X, op=ALU.add,
            )
            qmTh = wp.tile([128, HC], f8, tag="qmTh")
            nc.scalar.activation(qmTh, qmT, AF.Copy, scale=QM_SCALE / SEQ)

            # --- temporal MLP -> memT [F, K], memTh f16 -------------------
            h1T = wp.tile([F // 4, K], f32, tag="h1T")
            nc.vector.tensor_scalar_mul(h1T, tb_sb, Wt1T_sb)
            nc.vector.tensor_scalar_add(h1T, h1T, bt1T_sb)
            nc.vector.tensor_relu(h1T, h1T)
            tT_ps = pps.tile([F, K], f32, tag="tT")
            nc.tensor.matmul(tT_ps, lhsT=Wt2_sb, rhs=h1T, start=True, stop=True)
            memT_sb = wp.tile([F, K], f32, tag="memT")
            nc.scalar.activation(memT_sb, tT_ps, AF.Identity, bias=bt2T_sb,
                                 scale=1.0)
            nc.vector.tensor_add(memT_sb, memT_sb, dgT_sb)
            memTh_sb = wp.tile([F, K], f16, tag="memTh")
            nc.vector.tensor_copy(memTh_sb, memT_sb)

            # --- gT[p, c, k] = (mem_k @ Wg/K)[c*128+p]  (PE, 8 matmuls) ---
            gT_ps = ppb.tile([128, HC, K], f32, tag="gT")
            for c in range(HC):
                nc.tensor.matmul(gT_ps[:, c, :],
                                 lhsT=Wg_sb[:, c * 128 : (c + 1) * 128],
                                 rhs=memTh_sb, start=True, stop=True)
            gT_sb = wp.tile([128, HC, K], f32, tag="gTsb")
            nc.vector.tensor_copy(gT_sb, gT_ps)

            # --- hq[f] = Wa1q^T qmean  (8 fp8 matmuls, x QM_SCALE) --------
            hq_ps = pps.tile([F, 1], f32, tag="hq")
            for c in range(HC):
                nc.tensor.matmul(hq_ps, lhsT=apk_sb[:, c, :],
                                 rhs=qmTh[:, c : c + 1],
                                 start=(c == 0), stop=(c == HC - 1))

            # --- scorer: w = sigmoid(tanh(ha + hq + ba1) @ Wa2 + ba2) -----
            haT_ps = pps.tile([F, K], f32, tag="haT")
            nc.tensor.matmul(haT_ps, lhsT=Wa1m_sb, rhs=memTh_sb,
                             start=True, stop=True)
            hqb = wp.tile([F, 1], f32, tag="hqb")
            # undo the fp8 pre-scale while adding ba1
            nc.scalar.activation(hqb, hq_ps, AF.Identity, bias=ba1T_sb,
                                 scale=1.0 / QM_SCALE)
            aT_sb = wp.tile([F, K], f32, tag="aT")
            nc.scalar.activation(aT_sb, haT_ps, AF.Tanh, bias=hqb, scale=1.0)
            score_ps = pps.tile([1, K], f32, tag="score")
            nc.tensor.matmul(score_ps, lhsT=Wa2_sb, rhs=aT_sb,
                             start=True, stop=True)
            wvT = wp.tile([1, K], f32, tag="wvT")
            nc.scalar.activation(wvT, score_ps, AF.Sigmoid, bias=ba2b_sb,
                                 scale=1.0)
            wvb = wp.tile([128, K], f32, tag="wvb")
            nc.gpsimd.partition_broadcast(wvb[:, :], wvT[:, :])
            wsum = wp.tile([128, 1], f32, tag="wsum")
            nc.vector.tensor_add(wsum, wvb[:, 0:1], wvb[:, 1:2])

            # --- G = w0 g0 + w1 g1 + (w0+w1) bg/K, in f16 -----------------
            G0 = wp.tile([128, HC], f32, tag="G0")
            nc.vector.tensor_scalar_mul(G0, gT_sb[:, :, 0], wvb[:, 0:1])
            G1 = wp.tile([128, HC], f32, tag="G1")
            nc.vector.scalar_tensor_tensor(G1, gT_sb[:, :, 1], wvb[:, 1:2], G0,
                                           ALU.mult, ALU.add)
            Gf = wp.tile([128, HC], f32, tag="Gf")
            nc.vector.scalar_tensor_tensor(Gf, bgT_sb, wsum, G1,
                                           ALU.mult, ALU.add)
            Gh = wp.tile([128, HC], f16, tag="Gh")
            nc.vector.tensor_copy(Gh, Gf)

            # --- matvec row[s] = G . key[s]  (PE, 16 matmuls) -------------
            row_ps = ppb.tile([1, SHARD], f32, tag="row")
            for c in range(HC):
                for hh in range(2):
                    nc.tensor.matmul(
                        row_ps[:, hh * 512 : (hh + 1) * 512],
                        lhsT=Gh[:, c : c + 1],
                        rhs=k_ap(c, hh),
                        start=(c == 0),
                        stop=(c == HC - 1),
                    )

            orow_sb = wp.tile([1, SHARD], f32, tag="orow")
            nc.scalar.copy(orow_sb[:, 0:512], row_ps[:, 0:512])
            nc.vector.tensor_copy(orow_sb[:, 512:1024], row_ps[:, 512:1024])
            nc.gpsimd.dma_start(orow.rearrange("(a b) -> a b", a=1), orow_sb)

    nc.compile()
    return nc


def _get_prog(which):
    if which not in _PROG_CACHE:
        _PROG_CACHE[which] = _build_p1() if which == 1 else _build_p2()
    return _PROG_CACHE[which]


def _p1_in_maps(inputs):
    import ml_dtypes

    f8 = ml_dtypes.float8_e4m3
    q = np.asarray(inputs["query"], np.float32).reshape(SEQ, H)
    in_maps = []
    for d in range(NCORES):
        qsh = q[d * SHARD : (d + 1) * SHARD]  # [1024 s, 1024 h]
        # qT[p, c, s] = q[s, 8p + c]
        qTd = np.ascontiguousarray(
            qsh.T.reshape(128, HC, SHARD).astype(f8)
        )
        in_maps.append({"qT": qTd})
    return in_maps


def _p2_in_maps(inputs, qmTs):
    import ml_dtypes

    f8 = ml_dtypes.float8_e4m3
    k = np.asarray(inputs["key"], np.float32).reshape(SEQ, H)

    fpk = np.zeros((128, _F32_COLS), np.float32)
    # qpc[p, c*8 + d] = qmT_d[p, c]
    fpk[:, _C_QPC : _C_QPC + 64] = (
        np.stack(qmTs, axis=-1).reshape(128, 64)
    )
    fpk[:, _C_BT2] = np.asarray(inputs["bt2"], np.float32)
    fpk[:, _C_BA1] = np.asarray(inputs["ba1"], np.float32)
    fpk[:, _C_WA2] = np.asarray(inputs["Wa2"], np.float32).reshape(F)
    fpk[:, _C_DGT : _C_DGT + K] = np.asarray(inputs["dg_features"], np.float32).T
    fpk[0, _C_BA2] = np.asarray(inputs["ba2"], np.float32)[0]
    fpk[0:32, _C_TB : _C_TB + K] = np.tile(
        np.asarray(inputs["timestamps"], np.float32)[None, :], (32, 1)
    )
    fpk[0:32, _C_WT1] = np.asarray(inputs["Wt1"], np.float32).reshape(F // 4)
    fpk[0:32, _C_BT1] = np.asarray(inputs["bt1"], np.float32)
    fpk[:, _C_BGT : _C_BGT + HC] = (
        np.asarray(inputs["bg"], np.float32) / K
    ).reshape(HC, 128).T
    fpk[0:32, _C_WT2 : _C_WT2 + F] = np.asarray(inputs["Wt2"], np.float32)

    Wa1 = np.asarray(inputs["Wa1"], np.float32)
    apk = np.ascontiguousarray(
        Wa1[F : F + H].reshape(128, HC, F).astype(f8)
    )

    hpk = np.zeros((128, _F16_COLS), np.float16)
    hpk[:, _C_WG : _C_WG + H] = (
        np.asarray(inputs["Wg"], np.float32) / K
    ).astype(np.float16)
    hpk[:, _C_WA1M : _C_WA1M + F] = Wa1[0:F].astype(np.float16)

    common = {"fpk": fpk, "apk": apk, "hpk": hpk,
              "wpk": np.ones((128, 64), np.float16)}
    in_maps = []
    for d in range(NCORES):
        ksh = k[d * SHARD : (d + 1) * SHARD]  # [1024 s, 1024 h]
        kTd = np.ascontiguousarray(
            ksh.T.reshape(HC, 128, SHARD).transpose(1, 0, 2).astype(np.float16)
        )
        m = dict(common)
        m["kT"] = kTd
        in_maps.append(m)
    return in_maps


def _run(inputs):
    from concourse.bass_utils import run_bass_kernel_spmd

    nc1 = _get_prog(1)
    res1 = run_bass_kernel_spmd(nc1, _p1_in_maps(inputs),
                                core_ids=list(range(NCORES)))
    qmTs = [res1.results[d]["qmT"] for d in range(NCORES)]

    nc2 = _get_prog(2)
    res2 = run_bass_kernel_spmd(nc2, _p2_in_maps(inputs, qmTs),
                                core_ids=list(range(NCORES)))
    row = np.concatenate([res2.results[d]["orow"] for d in range(NCORES)])
    return np.broadcast_to(row[None, None, None, :], (1, 1, SEQ, SEQ))


def kernel(**inputs) -> np.ndarray:
    return _run(inputs)


# revision 23
# speedup vs baseline: 1.0338x; 1.0338x over previous
"""Trainium2 Bass kernel for nn_CA1AttentionGate (two-program, zero-collective).

Reference computation (B=1, S=8192, H=1024, F=128, K=2):
    temporal = relu(t @ Wt1 + bt1) @ Wt2 + bt2          [K,F]
    mem      = dg_features + temporal                    [K,F]
    qmean    = query.mean(axis=1)                        [1,H]
    score_k  = tanh([mem_k ; qmean] @ Wa1 + ba1) @ Wa2 + ba2
    w_k      = sigmoid(score_k)
    g_k      = mem_k @ Wg + bg                           [K,H]
    row[s]   = (1/K) * sum_k w_k * (g_k . key[s])        [S]
    out      = broadcast(row) -> [1,1,S,S]

Sharding: sequence-parallel across 8 cores.  Each core owns 1024 positions
of both query (for the qmean reduction) and key (for the gate row), and
produces only its 1024-wide slice of the broadcast row; the [S,S] broadcast
is a zero-copy numpy view at gather time (every row is identical).

Cross-core structure: the only global quantity is qmean.  A collective
would cost ~15us of modelled latency, so the kernel runs as TWO
back-to-back SPMD programs with a pure data-marshalling hop on the host:

  P1 (per core): read its query shard quantized to fp8 in a host-transposed
      h-major layout and reduce it over the sequence axis on DVE+ACT into
      per-column partial sums qmT[p, c] = sum_s q[s, 8p+c] (f32, 4KB out).
      (The PE would be 2.5x faster per element, but the cost model's
      p-state ramp pins DMA-gated matmul trains at the 1.2GHz MID clock,
      which makes the vector engines the better reducers here.)
  host: restack the 8 partial tiles (no arithmetic) into P2's input pack.
  P2 (per core): finish qmean + the scorer MLP -> w_k on device, build the
      combined gate vector G = sum_k w_k g_k (+ (sum w_k) bg), then do the
      matvec row[s] = G . key[s] over the host-pre-transposed f16 key
      shard on the PE, and write the 4KB row slice.

Quantization (tolerance is 2e-2; measured end-to-end error ~5e-4):
  query fp8 (qmean averages the noise down by sqrt(8192)), key/Wg/Wa1m
  f16, Wa1q fp8 x qmean fp8 (qmean pre-scaled by 64 so its ~0.01-scale
  values clear the fp8 subnormal floor; the 1/64 is folded back in the
  +ba1 bias step).  The output row stays f32.
"""

import numpy as np

SEQ = 8192
H = 1024
F = 128
K = 2
NCORES = 8
SHARD = SEQ // NCORES  # 1024
HC = H // 128  # 8 h-chunks

_PROG_CACHE = {}

QM_SCALE = 64.0  # qmean pre-scale feeding the fp8 hq matmuls

# f32 pack column layout (P2): qpc | small weights
_C_QPC = 0  # 64 cols: qpc[p, c*8+d] = qmT_d[p, c]
_C_BT2 = 64
_C_BA1 = 65
_C_WA2 = 66
_C_DGT = 67  # 2 cols
_C_BA2 = 69  # row 0 only
_C_TB = 70  # 2 cols, rows 0:32
_C_WT1 = 72  # rows 0:32
_C_BT1 = 73  # rows 0:32
_C_BGT = 74  # 8 cols
_C_WT2 = 82  # 128 cols, rows 0:32
_F32_COLS = 210

# f16 pack column layout (P2)
_C_WG = 0  # 1024 cols
_C_WA1M = 1024  # 128 cols
_F16_COLS = 1152

# P1 chunking: c-columns per DMA chunk (small first chunk for an early
# engine start, small last chunk for a short tail)
_P1_CHUNKS = (1, 2, 2, 2, 1)
# P2 kT chunking
_P2_CHUNKS = (3, 2, 2, 1)


def _build_p1():
    """P1: query-shard column sums on DVE+ACT.

    in : qT  fp8 [128, HC, SHARD]  (qT[p, c, s] = q_shard[s, 8p + c])
    out: qmT f32 [128, HC]         (qmT[p, c] = sum_s q_shard[s, 8p + c])
    """
    import concourse.bacc as bacc
    import concourse.tile as tile
    from concourse import mybir

    AF = mybir.ActivationFunctionType
    ALU = mybir.AluOpType
    f32 = mybir.dt.float32
    f8 = mybir.dt.float8e4

    nc = bacc.Bacc("TRN2", target_bir_lowering=False, debug=False,
                   num_devices=NCORES)

    qT = nc.dram_tensor("qT", [128, HC, SHARD], f8, kind="ExternalInput").ap()
    qmT_d = nc.dram_tensor("qmT", [128, HC], f32, kind="ExternalOutput").ap()

    with tile.TileContext(nc) as tc:
        with (
            tc.tile_pool(name="consts", bufs=1) as cp,
            tc.tile_pool(name="qstream", bufs=1) as qp,
        ):
            # --- stream the transposed query shard (sync queue) -----------
            qtiles = []
            pos = 0
            for i, w in enumerate(_P1_CHUNKS):
                qt = qp.tile([128, w * SHARD], f8, tag=f"q{i}")
                nc.sync.dma_start(
                    qt, qT.rearrange("p c s -> p (c s)")[:, pos * SHARD : (pos + w) * SHARD]
                )
                qtiles.append((qt, pos, w))
                pos += w

            out_sb = cp.tile([128, HC], f32, tag="qmT")

            # --- per-column sequence reduction on DVE + ACT ---------------
            def col_ap(c):
                for qt, pos, w in qtiles:
                    if pos <= c < pos + w:
                        return qt[:, (c - pos) * SHARD : (c - pos + 1) * SHARD]
                raise AssertionError

            junk = cp.tile([128, SHARD], f32, tag="junk")
            # DVE is the faster reducer (no accumulator-read tax), so it
            # takes 4 full columns + the first half of c7; ACT takes 3 full
            # columns + the second half of c7.
            for c in (0, 2, 4, 6):
                nc.vector.tensor_reduce(out_sb[:, c : c + 1], col_ap(c),
                                        axis=mybir.AxisListType.X, op=ALU.add)
            for c in (1, 3, 5):
                nc.scalar.activation(junk, col_ap(c), AF.Copy,
                                     accum_out=out_sb[:, c : c + 1])
            hparts = cp.tile([128, 2], f32, tag="hparts")
            src7 = col_ap(7)
            nc.vector.tensor_reduce(hparts[:, 0:1], src7[:, 0 : SHARD // 2],
                                    axis=mybir.AxisListType.X, op=ALU.add)
            nc.scalar.activation(junk[:, 0 : SHARD // 2],
                                 src7[:, SHARD // 2 : SHARD], AF.Copy,
                                 accum_out=hparts[:, 1:2])
            nc.vector.tensor_add(out_sb[:, 7:8], hparts[:, 0:1], hparts[:, 1:2])

            nc.sync.dma_start(qmT_d, out_sb)

    nc.compile()
    return nc


def _build_p2():
    """P2: qmean finish + scorer + gate row matvec.

    in : wpk f16 [128, 64]         (ones: PE/ACT warmup fodder)
         fpk f32 [128, _F32_COLS]  (qpc + packed small weights, see _C_*)
         apk fp8 [128, HC, F]      (Wa1[F:F+H] rows h=8i+c -> [i, c, f])
         hpk f16 [128, _F16_COLS]  (Wg/K | Wa1m)
         kT  f16 [128, HC, SHARD]  (kT[p, c, s] = key[d*SHARD+s, c*128+p])
    out: orow f32 [SHARD]
    """
    import concourse.bacc as bacc
    import concourse.tile as tile
    from concourse import mybir

    AF = mybir.ActivationFunctionType
    ALU = mybir.AluOpType
    f32 = mybir.dt.float32
    f16 = mybir.dt.float16
    f8 = mybir.dt.float8e4

    nc = bacc.Bacc("TRN2", target_bir_lowering=False, debug=False,
                   num_devices=NCORES)

    wpk = nc.dram_tensor("wpk", [128, 64], f16, kind="ExternalInput").ap()
    fpk = nc.dram_tensor("fpk", [128, _F32_COLS], f32, kind="ExternalInput").ap()
    apk = nc.dram_tensor("apk", [128, HC, F], f8, kind="ExternalInput").ap()
    hpk = nc.dram_tensor("hpk", [128, _F16_COLS], f16, kind="ExternalInput").ap()
    kT = nc.dram_tensor("kT", [128, HC, SHARD], f16, kind="ExternalInput").ap()
    orow = nc.dram_tensor("orow", [SHARD], f32, kind="ExternalOutput").ap()

    with tile.TileContext(nc) as tc:
        with (
            tc.tile_pool(name="consts", bufs=1) as cp,
            tc.tile_pool(name="work", bufs=1) as wp,
            tc.tile_pool(name="ps_small", bufs=1, space="PSUM") as pps,
            tc.tile_pool(name="ps_big", bufs=1, space="PSUM") as ppb,
        ):
            # --- the wire: tiny warmup fodder first, then fpk (feeds the
            # longest scorer chain, so it rides the ACT queue to land right
            # after wpk), then hpk/apk and the key stream on the sync queue
            wpk_sb = cp.tile([128, 64], f16, tag="wpk")
            nc.sync.dma_start(wpk_sb, wpk)
            fpk_sb = cp.tile([128, _F32_COLS], f32, tag="fpk")
            nc.scalar.dma_start(fpk_sb, fpk)
            hpk_sb = cp.tile([128, _F16_COLS], f16, tag="hpk")
            nc.sync.dma_start(hpk_sb, hpk)
            apk_sb = cp.tile([128, HC, F], f8, tag="apk")
            nc.sync.dma_start(apk_sb, apk)
            ktiles = []
            pos = 0
            for i, w in enumerate(_P2_CHUNKS):
                kt = cp.tile([128, w * SHARD], f16, tag=f"k{i}")
                nc.sync.dma_start(
                    kt, kT.rearrange("p c s -> p (c s)")[:, pos * SHARD : (pos + w) * SHARD]
                )
                ktiles.append((kt, pos, w))
                pos += w

            def k_ap(c, hh):
                for kt, p0, w in ktiles:
                    if p0 <= c < p0 + w:
                        base = (c - p0) * SHARD + hh * 512
                        return kt[:, base : base + 512]
                raise AssertionError

            # pack slices
            Wg_sb = hpk_sb[:, _C_WG : _C_WG + H]
            Wa1m_sb = hpk_sb[:, _C_WA1M : _C_WA1M + F]
            ones16 = wpk_sb[:, 0:1]
            warm_sb = wpk_sb[:, :]
            qpc_sb = fpk_sb[:, _C_QPC : _C_QPC + 64]
            bt2T_sb = fpk_sb[:, _C_BT2 : _C_BT2 + 1]
            ba1T_sb = fpk_sb[:, _C_BA1 : _C_BA1 + 1]
            Wa2_sb = fpk_sb[:, _C_WA2 : _C_WA2 + 1]
            dgT_sb = fpk_sb[:, _C_DGT : _C_DGT + K]
            ba2b_sb = fpk_sb[0:1, _C_BA2 : _C_BA2 + 1]
            bgT_sb = fpk_sb[:, _C_BGT : _C_BGT + HC]
            tb_sb = fpk_sb[0:32, _C_TB : _C_TB + K]
            Wt1T_sb = fpk_sb[0:32, _C_WT1 : _C_WT1 + 1]
            bt1T_sb = fpk_sb[0:32, _C_BT1 : _C_BT1 + 1]
            Wt2_sb = fpk_sb[0:32, _C_WT2 : _C_WT2 + F]

            # --- ACT table warmup for the late tanh/sigmoid ---------------
            w1 = cp.tile([1, 1], f32, tag="w1")
            nc.scalar.activation(w1, wpk_sb[0:1, 0:1], AF.Tanh)
            w2 = cp.tile([1, 1], f32, tag="w2")
            nc.scalar.activation(w2, w1, AF.Sigmoid)

            # --- PE p-state warmup fodder ---------------------------------
            wjunk = pps.tile([1, 64], f32, tag="wjunk")
            for _ in range(10):
                nc.tensor.matmul(wjunk, lhsT=ones16, rhs=warm_sb,
                                 start=True, stop=True)

            # --- qmean columns (x QM_SCALE/SEQ, in fp8 for the hq matmuls)
            qmT = wp.tile([128, HC], f32, tag="qmT")
            nc.vector.tensor_reduce(
                qmT, qpc_sb.rearrange("p (c d) -> p c d", c=HC),
                axis=mybir.AxisListType.X, op=ALU.add,
            )
            qmTh = wp.tile([128, HC], f8, tag="qmTh")
            nc.scalar.activation(qmTh, qmT, AF.Copy, scale=QM_SCALE / SEQ)

            # --- temporal MLP -> memT [F, K], memTh f16 -------------------
            h1T = wp.tile([F // 4, K], f32, tag="h1T")
            nc.vector.tensor_scalar_mul(h1T, tb_sb, Wt1T_sb)
            nc.vector.tensor_scalar_add(h1T, h1T, bt1T_sb)
            nc.vector.tensor_relu(h1T, h1T)
            tT_ps = pps.tile([F, K], f32, tag="tT")
            nc.tensor.matmul(tT_ps, lhsT=Wt2_sb, rhs=h1T, start=True, stop=True)
            memT_sb = wp.tile([F, K], f32, tag="memT")
            nc.scalar.activation(memT_sb, tT_ps, AF.Identity, bias=bt2T_sb,
                                 scale=1.0)
            nc.vector.tensor_add(memT_sb, memT_sb, dgT_sb)
            memTh_sb = wp.tile([F, K], f16, tag="memTh")
            nc.vector.tensor_copy(memTh_sb, memT_sb)

            # --- gT[p, c, k] = (mem_k @ Wg/K)[c*128+p]  (PE, 8 matmuls) ---
            gT_ps = ppb.tile([128, HC, K], f32, tag="gT")
            for c in range(HC):
                nc.tensor.matmul(gT_ps[:, c, :],
                                 lhsT=Wg_sb[:, c * 128 : (c + 1) * 128],
                                 rhs=memTh_sb, start=True, stop=True)
            gT_sb = wp.tile([128, HC, K], f32, tag="gTsb")
            nc.vector.tensor_copy(gT_sb, gT_ps)

            # --- hq[f] = Wa1q^T qmean  (8 fp8 matmuls, x QM_SCALE) --------
            hq_ps = pps.tile([F, 1], f32, tag="hq")
            for c in range(HC):
                nc.tensor.matmul(hq_ps, lhsT=apk_sb[:, c, :],
                                 rhs=qmTh[:, c : c + 1],
                                 start=(c == 0), stop=(c == HC - 1))

            # --- scorer: w = sigmoid(tanh(ha + hq + ba1) @ Wa2 + ba2) -----
            haT_ps = pps.tile([F, K], f32, tag="haT")
            nc.tensor.matmul(haT_ps, lhsT=Wa1m_sb, rhs=memTh_sb,
                             start=True, stop=True)
            hqb = wp.tile([F, 1], f32, tag="hqb")
            # undo the fp8 pre-scale while adding ba1
            nc.scalar.activation(hqb, hq_ps, AF.Identity, bias=ba1T_sb,
                                 scale=1.0 / QM_SCALE)
            aT_sb = wp.tile([F, K], f32, tag="aT")
            nc.scalar.activation(aT_sb, haT_ps, AF.Tanh, bias=hqb, scale=1.0)
            score_ps = pps.tile([1, K], f32, tag="score")
            nc.tensor.matmul(score_ps, lhsT=Wa2_sb, rhs=aT_sb,
                             start=True, stop=True)
            wvT = wp.tile([1, K], f32, tag="wvT")
            nc.scalar.activation(wvT, score_ps, AF.Sigmoid, bias=ba2b_sb,
                                 scale=1.0)
            wvb = wp.tile([128, K], f32, tag="wvb")
            nc.gpsimd.partition_broadcast(wvb[:, :], wvT[:, :])
            wsum = wp.tile([128, 1], f32, tag="wsum")
            nc.vector.tensor_add(wsum, wvb[:, 0:1], wvb[:, 1:2])

            # --- G = w0 g0 + w1 g1 + (w0+w1) bg/K, in f16 -----------------
            G0 = wp.tile([128, HC], f32, tag="G0")
            nc.vector.tensor_scalar_mul(G0, gT_sb[:, :, 0], wvb[:, 0:1])
            G1 = wp.tile([128, HC], f32, tag="G1")
            nc.vector.scalar_tensor_tensor(G1, gT_sb[:, :, 1], wvb[:, 1:2], G0,
                                           ALU.mult, ALU.add)
            Gf = wp.tile([128, HC], f32, tag="Gf")
            nc.vector.scalar_tensor_tensor(Gf, bgT_sb, wsum, G1,
                                           ALU.mult, ALU.add)
            Gh = wp.tile([128, HC], f16, tag="Gh")
            nc.vector.tensor_copy(Gh, Gf)

            # --- matvec row[s] = G . key[s]  (PE, 16 matmuls) -------------
            row_ps = ppb.tile([1, SHARD], f32, tag="row")
            for c in range(HC):
                for hh in range(2):
                    nc.tensor.matmul(
                        row_ps[:, hh * 512 : (hh + 1) * 512],
                        lhsT=Gh[:, c : c + 1],
                        rhs=k_ap(c, hh),
                        start=(c == 0),
                        stop=(c == HC - 1),
                    )

            orow_sb = wp.tile([1, SHARD], f32, tag="orow")
            nc.scalar.copy(orow_sb[:, 0:512], row_ps[:, 0:512])
            nc.vector.tensor_copy(orow_sb[:, 512:1024], row_ps[:, 512:1024])
            nc.sync.dma_start(orow.rearrange("(a b) -> a b", a=1), orow_sb)

    nc.compile()
    return nc


def _get_prog(which):
    if which not in _PROG_CACHE:
        _PROG_CACHE[which] = _build_p1() if which == 1 else _build_p2()
    return _PROG_CACHE[which]


def _p1_in_maps(inputs):
    import ml_dtypes

    f8 = ml_dtypes.float8_e4m3
    q = np.asarray(inputs["query"], np.float32).reshape(SEQ, H)
    in_maps = []
    for d in range(NCORES):
        qsh = q[d * SHARD : (d + 1) * SHARD]  # [1024 s, 1024 h]
        # qT[p, c, s] = q[s, 8p + c]
        qTd = np.ascontiguousarray(
            qsh.T.reshape(128, HC, SHARD).astype(f8)
        )
        in_maps.append({"qT": qTd})
    return in_maps


def _p2_in_maps(inputs, qmTs):
    import ml_dtypes

    f8 = ml_dtypes.float8_e4m3
    k = np.asarray(inputs["key"], np.float32).reshape(SEQ, H)

    fpk = np.zeros((128, _F32_COLS), np.float32)
    # qpc[p, c*8 + d] = qmT_d[p, c]
    fpk[:, _C_QPC : _C_QPC + 64] = (
        np.stack(qmTs, axis=-1).reshape(128, 64)
    )
    fpk[:, _C_BT2] = np.asarray(inputs["bt2"], np.float32)
    fpk[:, _C_BA1] = np.asarray(inputs["ba1"], np.float32)
    fpk[:, _C_WA2] = np.asarray(inputs["Wa2"], np.float32).reshape(F)
    fpk[:, _C_DGT : _C_DGT + K] = np.asarray(inputs["dg_features"], np.float32).T
    fpk[0, _C_BA2] = np.asarray(inputs["ba2"], np.float32)[0]
    fpk[0:32, _C_TB : _C_TB + K] = np.tile(
        np.asarray(inputs["timestamps"], np.float32)[None, :], (32, 1)
    )
    fpk[0:32, _C_WT1] = np.asarray(inputs["Wt1"], np.float32).reshape(F // 4)
    fpk[0:32, _C_BT1] = np.asarray(inputs["bt1"], np.float32)
    fpk[:, _C_BGT : _C_BGT + HC] = (
        np.asarray(inputs["bg"], np.float32) / K
    ).reshape(HC, 128).T
    fpk[0:32, _C_WT2 : _C_WT2 + F] = np.asarray(inputs["Wt2"], np.float32)

    Wa1 = np.asarray(inputs["Wa1"], np.float32)
    apk = np.ascontiguousarray(
        Wa1[F : F + H].reshape(128, HC, F).astype(f8)
    )

    hpk = np.zeros((128, _F16_COLS), np.float16)
    hpk[:, _C_WG : _C_WG + H] = (
        np.asarray(inputs["Wg"], np.float32) / K
    ).astype(np.float16)
    hpk[:, _C_WA1M : _C_WA1M + F] = Wa1[0:F].astype(np.float16)

    common = {"fpk": fpk, "apk": apk, "hpk": hpk,
              "wpk": np.ones((128, 64), np.float16)}
    in_maps = []
    for d in range(NCORES):
        ksh = k[d * SHARD : (d + 1) * SHARD]  # [1024 s, 1024 h]
        kTd = np.ascontiguousarray(
            ksh.T.reshape(HC, 128, SHARD).transpose(1, 0, 2).astype(np.float16)
        )
        m = dict(common)
        m["kT"] = kTd
        in_maps.append(m)
    return in_maps


def _run(inputs):
    from concourse.bass_utils import run_bass_kernel_spmd

    nc1 = _get_prog(1)
    res1 = run_bass_kernel_spmd(nc1, _p1_in_maps(inputs),
                                core_ids=list(range(NCORES)))
    qmTs = [res1.results[d]["qmT"] for d in range(NCORES)]

    nc2 = _get_prog(2)
    res2 = run_bass_kernel_spmd(nc2, _p2_in_maps(inputs, qmTs),
                                core_ids=list(range(NCORES)))
    row = np.concatenate([res2.results[d]["orow"] for d in range(NCORES)])
    return np.broadcast_to(row[None, None, None, :], (1, 1, SEQ, SEQ))


def kernel(**inputs) -> np.ndarray:
    return _run(inputs)
